# revision 18
# baseline (speedup 1.0000x reference)
"""Causal self-attention (B=4, T=2048, D=1024, H=16) on 8 TRN2 NeuronCores.

Sharding: core i = (batch b = i//2, head-group g = i%2). Data parallel on B,
tensor parallel on heads (8 heads per group): qkv_proj columns and out_proj
rows split per head group. Each core computes a partial [D, T] output^T for
its batch; host sums the two group partials per batch, transposes, adds bias.

v2: all matmul operands in bfloat16 (rel err ~3.9e-3 vs 2e-2 budget).
 - x is transposed and cast to bf16 on the HOST: no PE transposes at all.
 - Q^T/K^T stay resident in SBUF (bf16 halves them): no DRAM bounce.
 - 1/sqrt(64) folded into the exp's scale immediate.
 - bf16 streams at 1 cycle/row for every N (fp32r needed N>=256).

v2c: single interleaved emission stream. Phase 2 runs qc-major; the
projection groups for t-chunk c+1 and the output-projection groups for
t-chunk c-1 are woven between AV runs of q-chunk c, so the in-order PE
queue always holds exp-independent work while ACT catches up on the
softmax exponentials.

Per-core pipeline:
  proj (per 512 t-chunk): V = xT.T@Wv (natural [t,d] + ones col into vS);
        Q^T, K^T = W.T @ xT directly in [d, t] layout, resident in SBUF.
  attn (per head pair p, q-chunk qc, k-tile j): S^T[k,q] = K^T.T @ Q^T
        (heads at partitions 0-63 / 64-127); one exp (scale=0.125) over both
        heads' strips trimmed to the causal columns; triangle mask-mul on
        the diagonal 128-block; AV: psum[65,512] += V'[k,d+1].T @ P^T over
        j -- row 64 is the softmax denominator (ones column). Normalize via
        reciprocal_approx_fast + gpsimd partition_broadcast.
  out  (per t-chunk, per 128 f-tile): out^T[f,t] = sum_p Wo_p[d,f].T @ O^T_p.
"""

import numpy as np
import ml_dtypes

import concourse.bacc as bacc
import concourse.tile as tile
import concourse.mybir as mybir
from concourse import bass_utils
from concourse.bass import ts

F32 = mybir.dt.float32
BF16 = mybir.dt.bfloat16
EXP = mybir.ActivationFunctionType.Exp
NP_BF16 = ml_dtypes.bfloat16

T = 2048
TT = 16          # t tiles of 128
NP = 4           # head pairs per core
NQC = 4          # q chunks of 512
SCALE = 0.125    # 1/sqrt(64), folded into the exp scale

_CACHE = {}
_last_in_maps = None


def _build(CT):
    """CT = number of 128-row c-tiles in the (possibly bias-augmented) x/W."""
    nc = bacc.Bacc("TRN2", target_bir_lowering=False, debug=False)
    C = CT * 128

    # emission order = per-engine execution order; keep the PE in same-type
    # runs (S K=64 vs AV/proj K=128) to amortize the array-reconfig cost
    def mm(*args, **kwargs):
        return nc.tensor.matmul(*args, **kwargs)

    xaT = nc.dram_tensor("xaT", [C, T], BF16, kind="ExternalInput").ap()
    wq = nc.dram_tensor("wq", [C, 512], BF16, kind="ExternalInput").ap()
    wk = nc.dram_tensor("wk", [C, 512], BF16, kind="ExternalInput").ap()
    wv = nc.dram_tensor("wv", [C, 512], BF16, kind="ExternalInput").ap()
    wo = nc.dram_tensor("wo", [512, 1024], BF16, kind="ExternalInput").ap()
    tri = nc.dram_tensor("tri", [128, 128], BF16, kind="ExternalInput").ap()
    ot = nc.dram_tensor("ot", [1024, T], BF16, kind="ExternalOutput").ap()

    with tile.TileContext(nc) as tc:
        with tc.tile_pool(name="persist", bufs=1) as persist:
            xT = persist.tile([128, CT, T], BF16)      # [c128, ctile, t]
            QT = persist.tile([128, NP, T], BF16)      # [d128(2 heads), pair, t]
            KT = persist.tile([128, NP, T], BF16)
            vS = persist.tile([128, TT, 8, 65], BF16)  # [k128, ktile, head, d+1]
            OT = persist.tile([128, NP, T], BF16)
            tr = persist.tile([128, 128], BF16)
            wo_sb = persist.tile([128, NP, 1024], BF16)
            wv_sb = persist.tile([128, CT, 512], BF16)
            wq_sb = persist.tile([128, CT, NP, 128], BF16)
            wk_sb = persist.tile([128, CT, NP, 128], BF16)
            nc.vector.memset(vS[:, :, :, 64:65], 1.0)

            with (
                tc.tile_pool(name="ptp", bufs=10) as ptpool,
                tc.tile_pool(name="rsm", bufs=3) as rpool,
                tc.tile_pool(name="rbcp", bufs=2) as rbcpool,
                tc.tile_pool(name="obnc", bufs=4) as opool,
                tc.tile_pool(name="psp", bufs=2, space="PSUM") as psp,
                tc.tile_pool(name="psS", bufs=2, space="PSUM") as psS,
                tc.tile_pool(name="psAv", bufs=2, space="PSUM") as psAv,
            ):
                # ---- DMA: first t-chunk and wv split per c-tile so the
                # first vproj matmuls start after ~1us of DMA; remaining
                # weights on the gpsimd queue in parallel ----
                xaT_r = xaT.rearrange("(ct P) t -> P ct t", P=128)
                wv_r = wv.rearrange("(ct P) f -> P ct f", P=128)
                wq_r = wq.rearrange("(ct P) (np f) -> P ct np f", P=128, np=NP)
                wk_r = wk.rearrange("(ct P) (np f) -> P ct np f", P=128, np=NP)
                for cc in range(CT):
                    nc.sync.dma_start(
                        out=xT[:, cc, 0:128], in_=xaT_r[:, cc, 0:128]
                    )
                    nc.gpsimd.dma_start(out=wv_sb[:, cc, :], in_=wv_r[:, cc, :])
                for cc in range(CT):
                    nc.sync.dma_start(
                        out=xT[:, cc, 128:512], in_=xaT_r[:, cc, 128:512]
                    )
                for cc in range(CT):
                    nc.gpsimd.dma_start(out=wq_sb[:, cc], in_=wq_r[:, cc])
                    nc.gpsimd.dma_start(out=wk_sb[:, cc], in_=wk_r[:, cc])
                for tc_ in range(1, 4):
                    nc.sync.dma_start(
                        out=xT[:, :, ts(tc_, 512)], in_=xaT_r[:, :, ts(tc_, 512)]
                    )
                nc.gpsimd.dma_start(out=tr, in_=tri)
                nc.gpsimd.dma_start(
                    out=wo_sb, in_=wo.rearrange("(np P) f -> P np f", P=128)
                )

                # ---- work-item definitions ----
                def vproj_tt(tt):
                    ps = psp.tile([128, 512], F32, name="ps", tag="ps")
                    for cc in range(CT):
                        mm(
                            ps,
                            lhsT=xT[:, cc, ts(tt, 128)],
                            rhs=wv_sb[:, cc, :],
                            start=(cc == 0),
                            stop=(cc == CT - 1),
                        )
                    nc.vector.tensor_copy(
                        out=vS[:, tt, :, 0:64],
                        in_=ps.rearrange("p (h d) -> p h d", h=8),
                    )

                def qk_item(p, w_sb, dst, tc_):
                    ps = psp.tile([128, 512], F32, name="ps", tag="ps")
                    for cc in range(CT):
                        mm(
                            ps,
                            lhsT=w_sb[:, cc, p, :],
                            rhs=xT[:, cc, ts(tc_, 512)],
                            start=(cc == 0),
                            stop=(cc == CT - 1),
                        )
                    nc.vector.tensor_copy(out=dst[:, p, ts(tc_, 512)], in_=ps)

                def ph1_items(c):
                    items = [
                        (lambda tt=tt: vproj_tt(tt))
                        for tt in range(4 * c, 4 * c + 4)
                    ]
                    for p in range(NP):
                        for w_sb, dst in ((wq_sb, QT), (wk_sb, KT)):
                            items.append(
                                lambda p=p, w=w_sb, d=dst, c=c: qk_item(p, w, d, c)
                            )
                    return items

                def out_item(c, ft):
                    ps = psp.tile([128, 512], F32, name="ps", tag="ps")
                    for p in range(NP):
                        mm(
                            ps,
                            lhsT=wo_sb[:, p, ts(ft, 128)],
                            rhs=OT[:, p, ts(c, 512)],
                            start=(p == 0),
                            stop=(p == NP - 1),
                        )
                    ob = opool.tile([128, 512], BF16, name="ob", tag="ob")
                    nc.vector.tensor_copy(out=ob, in_=ps)
                    nc.sync.dma_start(out=ot[ts(ft, 128), ts(c, 512)], in_=ob)

                # last t-chunk: p=0..2 partial runs as filler while the final
                # pair is still in flight; only the p=3 matmul + a DVE add
                # sit behind the last normalize
                o3p = persist.tile([128, 8, 512], BF16, name="o3p")

                def out3_partial(ft):
                    ps = psp.tile([128, 512], F32, name="ps", tag="ps")
                    for p in range(NP - 1):
                        mm(
                            ps,
                            lhsT=wo_sb[:, p, ts(ft, 128)],
                            rhs=OT[:, p, ts(3, 512)],
                            start=(p == 0),
                            stop=(p == NP - 2),
                        )
                    nc.vector.tensor_copy(out=o3p[:, ft, :], in_=ps)

                def out3_final(ft):
                    ps = psp.tile([128, 512], F32, name="ps", tag="ps")
                    mm(
                        ps,
                        lhsT=wo_sb[:, NP - 1, ts(ft, 128)],
                        rhs=OT[:, NP - 1, ts(3, 512)],
                        start=True,
                        stop=True,
                    )
                    ob = opool.tile([128, 512], BF16, name="ob", tag="ob")
                    nc.vector.tensor_add(ob, ps, o3p[:, ft, :])
                    nc.sync.dma_start(out=ot[ts(ft, 128), ts(3, 512)], in_=ob)

                avs = {}
                pts = {}

                def s_exp(p, qc, j):
                    off = max(0, 128 * j - 512 * qc)
                    sg = psS.tile([128, 2, 512], F32, name="sg", tag="sg")
                    jo = 128 * j
                    for m in range(2):
                        mm(
                            sg[:, m, off:],
                            lhsT=KT[64 * m : 64 * m + 64, p, jo : jo + 128],
                            rhs=QT[64 * m : 64 * m + 64, p, 512 * qc + off : 512 * qc + 512],
                            start=True,
                            stop=True,
                        )
                    ptile = ptpool.tile([128, 2, 512], BF16, name="ptile", tag="ptile")
                    nc.scalar.activation(
                        out=ptile[:, :, off:], in_=sg[:, :, off:], func=EXP,
                        scale=SCALE,
                    )
                    if j >= 4 * qc:
                        nc.vector.tensor_mul(
                            ptile[:, :, off : off + 128],
                            ptile[:, :, off : off + 128],
                            tr[:, None, :].to_broadcast([128, 2, 128]),
                        )
                    pts[(p, qc, j)] = (ptile, off)

                def av_mm(p, qc, j, nj):
                    ptile, off = pts.pop((p, qc, j))
                    av = avs[(p, qc)]
                    for m in range(2):
                        mm(
                            av[m][:65, off:],
                            lhsT=vS[:, j, 2 * p + m, :],
                            rhs=ptile[:, m, off:],
                            start=(j == 0),
                            stop=(j == nj - 1),
                        )

                def normalize(p, qc):
                    av = avs.pop((p, qc))
                    # denominator row out of PSUM first (reciprocal cannot
                    # read PSUM) so the recip+broadcast chain starts ASAP;
                    # the O~ copies then overlap the gpsimd broadcast
                    rsb = rpool.tile([1, 2, 512], F32, name="rsb", tag="rsb")
                    for m in range(2):
                        nc.vector.tensor_copy(out=rsb[:, m, :], in_=av[m][64:65, :])
                    rinv = rpool.tile([1, 2, 512], F32, name="rinv", tag="rinv")
                    nc.vector.reciprocal_approx_fast(out=rinv, in_=rsb)
                    for m in range(2):
                        nc.vector.tensor_copy(
                            out=OT[64 * m : 64 * m + 64, p, ts(qc, 512)],
                            in_=av[m][0:64, :],
                        )
                    rb = rbcpool.tile([128, 2, 512], F32, name="rb", tag="rb")
                    nc.gpsimd.partition_broadcast(rb, rinv)
                    for m in range(2):
                        sl = OT[64 * m : 64 * m + 64, p, ts(qc, 512)]
                        nc.vector.tensor_mul(sl, sl, rb[64 * m : 64 * m + 64, m, :])

                # ---- qc-major group stream with woven proj/out fillers ----
                groups = []
                chunk_first = {}
                for qc in range(NQC):
                    for p in range(NP):
                        nj = 4 * qc + 4
                        js = list(range(nj))
                        sub = [js[i : i + 3] for i in range(0, nj, 3)]
                        for gi, jg in enumerate(sub):
                            if qc not in chunk_first:
                                chunk_first[qc] = len(groups)
                            groups.append(
                                (p, qc, nj, jg, gi == 0, gi == len(sub) - 1)
                            )
                first_to_chunk = {v: k for k, v in chunk_first.items()}
                chunk_of_group = [g[1] for g in groups]

                filler_q = []

                def av_group(gi):
                    p, qc, nj, jg, first, last = groups[gi]
                    if first:
                        avs[(p, qc)] = [
                            psAv.tile([128, 512], F32, name="av", tag="av")
                            for _ in range(2)
                        ]
                    for j in jg:
                        av_mm(p, qc, j, nj)
                    if last:
                        normalize(p, qc)
                        if p == NP - 1 and qc < NQC - 1:
                            # OT t-chunk qc complete: output projection may go
                            filler_q.extend(
                                lambda c=qc, ft=ft: out_item(c, ft)
                                for ft in range(8)
                            )
                        if qc == NQC - 1 and p == NP - 2:
                            filler_q.extend(
                                lambda ft=ft: out3_partial(ft) for ft in range(8)
                            )
                        if qc == NQC - 1 and p == NP - 1:
                            filler_q.extend(
                                lambda ft=ft: out3_final(ft) for ft in range(8)
                            )

                # fillers per av-pair point, tuned so proj chunk c+1 and out
                # chunk c-1 drain evenly across q-chunk c's points; chunk 3
                # is back-loaded: ACT falls behind toward its end, so save
                # the fillers for the late points where AV stalls on exp
                RATE = {0: 3.0, 1: 3.4, 2: 2.5, 3: 0.0}
                RATE3_LATE = 3.4  # from point 10 of chunk 3's 12

                for it in ph1_items(0):
                    it()

                LAG = 2
                acc = 0.0
                pts_in_chunk = 0
                cur_chunk = -1
                for i in range(len(groups) + LAG):
                    # AV pair (and its normalize) BEFORE this iteration's S
                    # group: keeps the PSUM-drain copies ahead of the next
                    # tri-mask mul in the in-order DVE queue, so the av banks
                    # release without waiting on ACT
                    if i >= LAG and (i - LAG) % 2 == 1:
                        av_group(i - LAG - 1)
                        av_group(i - LAG)
                        c = chunk_of_group[i - LAG]
                        if c != cur_chunk:
                            cur_chunk, pts_in_chunk = c, 0
                        pts_in_chunk += 1
                        if c == 3 and pts_in_chunk >= 10:
                            acc += RATE3_LATE
                        else:
                            acc += RATE[c]
                        take = min(int(acc), len(filler_q))
                        for _ in range(take):
                            filler_q.pop(0)()
                        acc -= take
                    if i < len(groups):
                        if i in first_to_chunk:
                            c = first_to_chunk[i]
                            if c + 1 < NQC:
                                filler_q.extend(ph1_items(c + 1))
                        p, qc, nj, jg, first, last = groups[i]
                        for j in jg:
                            s_exp(p, qc, j)
                if len(groups) % 2 == 1:
                    av_group(len(groups) - 1)
                for it in filler_q:
                    it()

    nc.compile()
    return nc


def kernel(x, W_qkv, b_qkv, W_out, b_out):
    global _last_in_maps
    x = np.asarray(x, dtype=np.float32)
    W_qkv = np.asarray(W_qkv, dtype=np.float32)
    b_qkv = np.asarray(b_qkv, dtype=np.float32)
    W_out = np.asarray(W_out, dtype=np.float32)
    b_out = np.asarray(b_out, dtype=np.float32)
    B = x.shape[0]

    aug = bool(np.any(b_qkv))
    CT = 9 if aug else 8
    if CT not in _CACHE:
        _CACHE[CT] = _build(CT)
    nc = _CACHE[CT]

    # triangle keep-mask for the diagonal 128 block: [p, c] = 1 if c >= p
    tri = (np.arange(128)[None, :] >= np.arange(128)[:, None]).astype(NP_BF16)

    in_maps = []
    for core in range(8):
        b, g = core // 2, core % 2
        xaT = x[b].T
        if aug:
            pad = np.zeros((128, T), np.float32)
            pad[0, :] = 1.0
            xaT = np.concatenate([xaT, pad], axis=0)

        def wslice(col0):
            w = W_qkv[:, col0 + 512 * g : col0 + 512 * g + 512]
            if aug:
                extra = np.zeros((128, 512), np.float32)
                extra[0] = b_qkv[col0 + 512 * g : col0 + 512 * g + 512]
                w = np.concatenate([w, extra], axis=0)
            return np.ascontiguousarray(w).astype(NP_BF16)

        in_maps.append(
            {
                "xaT": np.ascontiguousarray(xaT).astype(NP_BF16),
                "wq": wslice(0),
                "wk": wslice(1024),
                "wv": wslice(2048),
                "wo": np.ascontiguousarray(
                    W_out[512 * g : 512 * g + 512, :]
                ).astype(NP_BF16),
                "tri": tri,
            }
        )

    _last_in_maps = in_maps
    res = bass_utils.run_bass_kernel_spmd(nc, in_maps, list(range(8))).results
    out = np.empty((B, T, 1024), np.float32)
    for b in range(B):
        acc = res[2 * b]["ot"].astype(np.float32) + res[2 * b + 1]["ot"].astype(
            np.float32
        )
        out[b] = acc.T + b_out[None, :]
    return out
